# revision 61
# baseline (speedup 1.0000x reference)
"""Multi-head self-attention Trainium2 kernel (8 NeuronCores).

Problem: x[4, 2048, 1024], H=16 heads, D=64. Sharding: core c handles
batch b = c // 2 and head-group hg = c % 2 (8 heads = 512 features).

Per-core math (F = 512 core-local features, T = 2048 tokens, C = 1024),
all matmul operands fp16 (PE runs fp16 at 1 cycle/row like bf16, with
~8x the mantissa), fp32 PSUM accumulation:

  QT = (Wq_s.T @ x_b.T) + bq_s          [F, T]   (feature-major)
  KT = same with Wk_s                    [F, T]
  V65 = [x_b @ Wv_s + bv_s | ones]       [T, 8*(64+1)] interleaved per head
  per (head pair, 512-query block):
    for each key tile kt (128 keys):
      scT[key, q] = KT_h.T @ QT_h        (two heads packed into the PE
                                          array via partition offsets 0/64)
      ex = exp(scT / 8)                  fp16 (no max subtraction: |s|<~2)
      pv[65, q] += V65_h[kt].T @ ex      (row 64 = softmax denominator)
    attnT_h[:, q] = pv[0:64] * (1/pv[64])  (recip on DVE, replicated
                                          across partitions via PE matmul)
  o_part = attnT.T @ Wo_s                [T, C]
Host: out[b] = o_part[2b] + o_part[2b+1] + bo.

The attention loop is ACT(exp)-bound (~1.15us per 128x1024 tile), so
everything else is software-pipelined into its slack: the PE-side
normalize of block i is emitted mid-block i+1, and the projections for
later head pairs plus the output projection are fed from a filler
queue at two insertion points per block.
"""

import sys

import numpy as np

if "/opt/trn_rl_repo" not in sys.path:
    sys.path.insert(0, "/opt/trn_rl_repo")

import concourse.bass as bass
import concourse.mybir as mybir
import concourse.tile as tile
from concourse import bacc

F32 = mybir.dt.float32
F16 = mybir.dt.float16
AF = mybir.ActivationFunctionType

# Full-problem constants
B, N, C, H, D = 4, 2048, 1024, 16, 64
NCORES = 8
NH = 8          # heads per core
F = NH * D      # 512 core-local features
SCALE = 1.0 / 8.0  # 1/sqrt(D)


def build_attention_kernel(tok=N, cin=C, nh=NH):
    """Build the per-core Bass program. Returns the finalized Bass object."""
    f = nh * D
    assert tok % 512 == 0 and cin % 128 == 0 and f % 128 == 0
    c_t = cin // 128       # contraction tiles for projections (8)
    f_t = f // 128         # feature tiles = head pairs (4)
    t_t = tok // 128       # token tiles (16)
    n_qb = tok // 512      # query blocks (4)
    n_ct = cin // 128      # output-proj column tiles (8)

    nc = bacc.Bacc("TRN2", target_bir_lowering=False, debug=False,
                   num_devices=NCORES)

    xT = nc.dram_tensor("xT", [cin, tok], F16, kind="ExternalInput").ap()
    wq = nc.dram_tensor("wq", [cin, f], F16, kind="ExternalInput").ap()
    wk = nc.dram_tensor("wk", [cin, f], F16, kind="ExternalInput").ap()
    wv = nc.dram_tensor("wv", [cin, f], F16, kind="ExternalInput").ap()
    bq = nc.dram_tensor("bq", [f, 1], F32, kind="ExternalInput").ap()
    bk = nc.dram_tensor("bk", [f, 1], F32, kind="ExternalInput").ap()
    bv = nc.dram_tensor("bv", [1, f], F16, kind="ExternalInput").ap()
    wo = nc.dram_tensor("wo", [f, cin], F16, kind="ExternalInput").ap()
    o_part = nc.dram_tensor("o_part", [tok, cin], F16,
                            kind="ExternalOutput").ap()

    with tile.TileContext(nc) as tc:
        from contextlib import ExitStack
        with ExitStack() as ctx:
            # ---- pools ----
            p_sm = ctx.enter_context(tc.tile_pool(name="p_sm", bufs=1))
            p_x = ctx.enter_context(tc.tile_pool(name="p_x", bufs=1))
            p_w = ctx.enter_context(tc.tile_pool(name="p_w", bufs=1))
            p_qk = ctx.enter_context(tc.tile_pool(name="p_qk", bufs=1))
            p_v = ctx.enter_context(tc.tile_pool(name="p_v", bufs=1))
            p_at = ctx.enter_context(tc.tile_pool(name="p_at", bufs=1))
            p_ex = ctx.enter_context(tc.tile_pool(name="p_ex", bufs=3))
            p_dn = ctx.enter_context(tc.tile_pool(name="p_dn", bufs=4))
            p_st = ctx.enter_context(tc.tile_pool(name="p_st", bufs=4))
            p_os = ctx.enter_context(tc.tile_pool(name="p_os", bufs=3))
            # 6 banks of score ping-pong (3-deep: the ACT backlog absorbs
            # the inserted oproj/normalize PE work) + 2 banks of pv
            # accumulators (1-block-deep: the single staging copy frees
            # them fast enough)
            ps_sc = ctx.enter_context(
                tc.tile_pool(name="ps_sc", bufs=3, space="PSUM"))
            ps_pv = ctx.enter_context(
                tc.tile_pool(name="ps_pv", bufs=2, space="PSUM"))

            # ---- constants / biases ----
            ones64 = p_sm.tile([1, 64], F16, tag="ones64", name="ones64")
            nc.vector.memset(ones64[:, :], 1.0)
            # selectors for the one-shot denominator replicate: out rows
            # 0:64 get head-e's reciprocal, rows 64:128 get head-o's
            sel_e = p_sm.tile([1, 128], F16, tag="sele", name="sel_e")
            sel_o = p_sm.tile([1, 128], F16, tag="selo", name="sel_o")
            nc.vector.memset(sel_e[:, :], 0.0)
            nc.vector.memset(sel_e[:, 0:64], 1.0)
            nc.vector.memset(sel_o[:, :], 0.0)
            nc.vector.memset(sel_o[:, 64:128], 1.0)
            onestok = p_sm.tile([1, 128], F16, tag="onestok", name="onestok")
            nc.vector.memset(onestok[:, :], 1.0)
            # bq/bk arrive host-prearranged as [128, f_t] so the DMA is one
            # contiguous transfer (a strided rearrange here would emit
            # thousands of 4-byte packets and jam the queues at startup)
            bqs = p_sm.tile([128, f_t], F32, tag="bqs", name="bqs")
            bks = p_sm.tile([128, f_t], F32, tag="bks", name="bks")
            bvs = p_sm.tile([1, f], F16, tag="bvs", name="bvs")
            nc.sync.dma_start(bqs[:, :], bq.rearrange("(p a) o -> p (a o)", p=128))
            nc.sync.dma_start(bks[:, :], bk.rearrange("(p a) o -> p (a o)", p=128))
            nc.sync.dma_start(bvs[:, :], bv[:, :])

            # ---- weight + x loads (DMA, overlapped with compute) ----
            wk_s = [p_w.tile([128, f], F16, tag=f"wk{i}", name=f"wk_s{i}")
                    for i in range(c_t)]
            wv_s = [p_w.tile([128, f], F16, tag=f"wv{i}", name=f"wv_s{i}")
                    for i in range(c_t)]
            wq_s = [p_w.tile([128, f], F16, tag=f"wq{i}", name=f"wq_s{i}")
                    for i in range(c_t)]
            xs = [p_x.tile([128, tok], F16, tag=f"x{i}", name=f"xs{i}")
                  for i in range(c_t)]
            wo_s = [p_w.tile([128, cin], F16, tag=f"wo{i}", name=f"wo_s{i}")
                    for i in range(f_t)]
            # interleave the first K-projection group's inputs (wk + x
            # token-chunk 0) so it can start as early as possible; wv
            # before the x tail so V projection isn't the last to unblock
            for i in range(c_t):
                nc.sync.dma_start(wk_s[i][:, :], wk[i * 128:(i + 1) * 128, :])
                nc.sync.dma_start(xs[i][:, 0:512], xT[i * 128:(i + 1) * 128, 0:512])
            for tch in range(1, tok // 512):
                ts = slice(tch * 512, (tch + 1) * 512)
                for i in range(c_t):
                    nc.sync.dma_start(xs[i][:, ts], xT[i * 128:(i + 1) * 128, ts])
                    if tch == 1:
                        nc.sync.dma_start(wv_s[i][:, :],
                                          wv[i * 128:(i + 1) * 128, :])
            for i in range(c_t):
                nc.sync.dma_start(wq_s[i][:, :], wq[i * 128:(i + 1) * 128, :])
            for i in range(f_t):
                nc.sync.dma_start(wo_s[i][:, :], wo[i * 128:(i + 1) * 128, :])

            # ---- persistent activations ----
            KT = [p_qk.tile([128, tok], F16, tag=f"kt{i}", name=f"KT{i}")
                  for i in range(f_t)]
            QT = [p_qk.tile([128, tok], F16, tag=f"qt{i}", name=f"QT{i}")
                  for i in range(f_t)]
            # V tiles padded to 128 weight columns per head (64 values +
            # ones column at 64 + don't-care) so LDWEIGHTS gets FWL
            V65 = [p_v.tile([128, nh * 128], F16, tag=f"v{i}", name=f"V65_{i}")
                   for i in range(t_t)]
            attnT = [p_at.tile([128, tok], F16, tag=f"at{i}", name=f"attnT{i}")
                     for i in range(f_t)]

            # ---- emission helpers ----
            def qk_group(w_s, dst, bias, ft, tch):
                """One projection group: 512 tokens x 128 features of Q or K."""
                ts = slice(tch * 512, (tch + 1) * 512)
                ps = ps_sc.tile([128, 1024], F32, tag="sc",
                                name=f"psqk_{dst[ft].name}_{tch}")
                for i in range(c_t):
                    nc.tensor.matmul(
                        ps[:, 0:512],
                        w_s[i][:, ft * 128:(ft + 1) * 128],
                        xs[i][:, ts],
                        start=(i == 0), stop=(i == c_t - 1))
                nc.vector.tensor_scalar_add(
                    dst[ft][:, ts], ps[:, 0:512], bias[:, ft:ft + 1])

            def v_group(gt):
                """One V group: 128 tokens, all 512 features + ones col."""
                tsl = slice(gt * 128, (gt + 1) * 128)
                psv = ps_sc.tile([128, 1024], F32, tag="sc", name=f"psv{gt}")
                for i in range(c_t):
                    nc.tensor.matmul(
                        psv[:, 0:512], xs[i][:, tsl], wv_s[i][:, :],
                        start=(i == 0), stop=False)
                nc.tensor.matmul(psv[:, 0:512], onestok[:, :], bvs[:, :],
                                 start=False, stop=True)
                v_dst = V65[gt].rearrange("p (h e) -> p h e", e=128)
                nc.vector.tensor_copy(
                    v_dst[:, :, 0:64],
                    psv[:, 0:512].rearrange("p (h e) -> p h e", e=64)[:, :, :])

            def oproj_group(qb, tt4, oc, in_loop=False):
                """Output projection for 128 tokens x 512 out-channels.

                Allocates from the sc pool: mid-block the DVE is idle, so
                the drain copy frees the slot in ~0.7us. (The pv pool must
                NOT be used here — its 2-block rotation depth is what
                keeps the next block's pv accumulators from waiting on the
                previous block's DVE normalize chain.)
                """
                tt = qb * 4 + tt4
                tsl = slice(tt * 128, (tt + 1) * 128)
                osl = slice(oc * 512, (oc + 1) * 512)
                po = ps_sc.tile([128, 1024], F32, tag="sc",
                                name=f"po{tt}_{oc}")
                posl = po[:, 0:512]
                for i in range(f_t):
                    nc.tensor.matmul(posl, attnT[i][:, tsl],
                                     wo_s[i][:, osl],
                                     start=(i == 0), stop=(i == f_t - 1))
                ob = p_os.tile([128, 512], F16, tag="os", name=f"ob{tt}_{oc}")
                nc.vector.tensor_copy(ob[:, :], posl)
                nc.sync.dma_start(o_part[tsl, osl], ob[:, :])

            # ---- PE warm-up: junk matmuls while input DMAs are in
            # flight, so the HAM clock gate reaches 2.4 GHz before the
            # real work starts ----
            junk = p_sm.tile([128, 512], F16, tag="junk", name="junk")
            nc.vector.memset(junk[:, :], 0.5)
            ps_junk = ps_sc.tile([128, 1024], F32, tag="sc", name="ps_junk")
            for _ in range(30):
                nc.tensor.matmul(ps_junk[:, 0:512], junk[:, 0:128],
                                 junk[:, :], start=True, stop=True)

            def qk_group2(w_s, dst, bias, fa, fb, tch):
                """Two interleaved projection groups: consecutive matmuls
                alternate PSUM banks and weight sets, so each weight load
                and pipe drain hides under the other group's stream."""
                ts = slice(tch * 512, (tch + 1) * 512)
                psA = ps_sc.tile([128, 1024], F32, tag="sc",
                                 name=f"psqkA_{dst[fa].name}_{tch}")
                psB = ps_sc.tile([128, 1024], F32, tag="sc",
                                 name=f"psqkB_{dst[fb].name}_{tch}")
                for i in range(c_t):
                    nc.tensor.matmul(
                        psA[:, 0:512], w_s[i][:, fa * 128:(fa + 1) * 128],
                        xs[i][:, ts], start=(i == 0), stop=(i == c_t - 1))
                    nc.tensor.matmul(
                        psB[:, 0:512], w_s[i][:, fb * 128:(fb + 1) * 128],
                        xs[i][:, ts], start=(i == 0), stop=(i == c_t - 1))
                nc.vector.tensor_scalar_add(
                    dst[fa][:, ts], psA[:, 0:512], bias[:, fa:fa + 1])
                nc.vector.tensor_scalar_add(
                    dst[fb][:, ts], psB[:, 0:512], bias[:, fb:fb + 1])

            def v_group2(ga, gb):
                psAB = []
                for g in (ga, gb):
                    ps = ps_sc.tile([128, 1024], F32, tag="sc",
                                    name=f"psv{g}")
                    psAB.append(ps)
                for i in range(c_t):
                    for g, ps in zip((ga, gb), psAB):
                        tsl = slice(g * 128, (g + 1) * 128)
                        nc.tensor.matmul(
                            ps[:, 0:512], xs[i][:, tsl], wv_s[i][:, :],
                            start=(i == 0), stop=False)
                for g, ps in zip((ga, gb), psAB):
                    nc.tensor.matmul(ps[:, 0:512], onestok[:, :], bvs[:, :],
                                     start=False, stop=True)
                for g, ps in zip((ga, gb), psAB):
                    v_dst = V65[g].rearrange("p (h e) -> p h e", e=128)
                    nc.vector.tensor_copy(
                        v_dst[:, :, 0:64],
                        ps[:, 0:512].rearrange("p (h e) -> p h e",
                                               e=64)[:, :, :])

            # ---- upfront phase 1: all projections, ordered to match DMA
            # arrival (K token-chunk-major first, then V, then Q) ----
            for gt in range(t_t):
                nc.vector.memset(V65[gt][:, :], 1.0)
            for tch in range(n_qb):
                qk_group2(wk_s, KT, bks, 0, 1, tch)
                qk_group2(wk_s, KT, bks, 2, 3, tch)
            for gt in range(0, t_t, 2):
                v_group2(gt, gt + 1)
            # only query-block 0's Q upfront; the rest streams into the
            # early attention blocks' slack (3-deep sc pool absorbs it)
            qk_group2(wq_s, QT, bqs, 0, 1, 0)
            qk_group2(wq_s, QT, bqs, 2, 3, 0)
            q_filler = [(p, tch) for tch in range(1, n_qb)
                        for p in range(f_t)]

            # filler queue: small output-projection units (~1.2us each)
            # that fit inside the attention loop's PE slack
            filler = []

            def emit_fill():
                if filler:
                    filler.pop(0)()

            # ================= Phase 2: attention =================
            pending = []  # deferred PE-side normalize of the previous block

            for qb in range(n_qb):
                qsl = slice(qb * 512, (qb + 1) * 512)
                for pair in range(f_t):
                    he, ho = 2 * pair, 2 * pair + 1
                    pv_e = ps_pv.tile([128, 512], F32, tag="pv",
                                      name=f"pv{pair}_{qb}e")
                    pv_o = ps_pv.tile([128, 512], F32, tag="pv",
                                      name=f"pv{pair}_{qb}o")
                    for kt in range(t_t):
                        if kt == 2 and q_filler:
                            p2, t2 = q_filler.pop(0)
                            qk_group(wq_s, QT, bqs, p2, t2)
                        if kt == 6:
                            # PE-side normalize of the previous block: one
                            # sc-pool allocation holds both heads' 1/den
                            # replicas, so only a single rotation slot is
                            # coupled to the DVE multiplies
                            if pending:
                                att, stage2, dninv2 = pending.pop()
                                rp = ps_sc.tile([128, 1024], F32, tag="sc",
                                                name=f"rp{pair}_{qb}")
                                nc.tensor.matmul(rp[:, 0:512], sel_e[:, :],
                                                 dninv2[:, 0:512],
                                                 start=True, stop=False)
                                nc.tensor.matmul(rp[:, 0:512], sel_o[:, :],
                                                 dninv2[:, 512:1024],
                                                 start=False, stop=True)
                                nc.vector.tensor_mul(att, stage2[:, :],
                                                     rp[:, 0:512])
                                pending = []
                            # attnT for qb-1 became final once pair3's
                            # part B ran: queue its output projection
                            if pair == 0 and qb >= 1:
                                for tt4 in range(4):
                                    for oc in range(2):
                                        filler.append(
                                            lambda q=qb - 1, t=tt4, o=oc:
                                            oproj_group(q, t, o, in_loop=True))
                        if kt == 9:
                            emit_fill()
                        if kt == 13:
                            emit_fill()
                        ksl = slice(kt * 128, (kt + 1) * 128)
                        sc = ps_sc.tile([128, 1024], F32, tag="sc",
                                        name=f"sc{pair}_{qb}_{kt}")
                        nc.tensor.matmul(sc[:, 0:512],
                                         KT[pair][0:64, ksl],
                                         QT[pair][0:64, qsl],
                                         start=True, stop=True)
                        nc.tensor.matmul(sc[:, 512:1024],
                                         KT[pair][64:128, ksl],
                                         QT[pair][64:128, qsl],
                                         start=True, stop=True)
                        ex = p_ex.tile([128, 1024], F16, tag="ex",
                                       name=f"ex{pair}_{qb}_{kt}")
                        nc.scalar.activation(ex[:, :], sc[:, :], AF.Exp,
                                             scale=SCALE)
                        nc.tensor.matmul(pv_e[:, :],
                                         V65[kt][:, he * 128:he * 128 + 128],
                                         ex[:, 0:512],
                                         start=(kt == 0), stop=(kt == t_t - 1))
                        nc.tensor.matmul(pv_o[:, :],
                                         V65[kt][:, ho * 128:ho * 128 + 128],
                                         ex[:, 512:1024],
                                         start=(kt == 0), stop=(kt == t_t - 1))

                    # part A (DVE only): stage both heads' unnormalized attn
                    # into one [128,512] tile (frees the pv banks fast) and
                    # compute both reciprocals in a single [2,512] pass
                    stage2 = p_st.tile([128, 512], F32, tag="st",
                                       name=f"st_{pair}_{qb}")
                    dnr2 = p_dn.tile([1, 1024], F32, tag="dnr",
                                     name=f"dnr_{pair}_{qb}")
                    nc.vector.tensor_copy(stage2[0:64, :], pv_e[0:64, :])
                    nc.vector.tensor_copy(dnr2[:, 0:512], pv_e[64:65, :])
                    nc.vector.tensor_copy(stage2[64:128, :], pv_o[0:64, :])
                    nc.vector.tensor_copy(dnr2[:, 512:1024], pv_o[64:65, :])
                    dninv32 = p_dn.tile([1, 1024], F32, tag="dn32",
                                        name=f"dn32_{pair}_{qb}")
                    nc.vector.reciprocal_approx_fast(
                        out=dninv32[:, :], in_=dnr2[:, :])
                    dninv2 = p_dn.tile([1, 1024], F16, tag="dn",
                                       name=f"dn{pair}_{qb}")
                    nc.vector.tensor_copy(dninv2[:, :], dninv32[:, :])
                    pending.append((attnT[pair][:, qsl], stage2, dninv2))

            if pending:
                att, stage2, dninv2 = pending.pop()
                rp = ps_sc.tile([128, 1024], F32, tag="sc", name="rp_last")
                nc.tensor.matmul(rp[:, 0:512], sel_e[:, :],
                                 dninv2[:, 0:512], start=True, stop=False)
                nc.tensor.matmul(rp[:, 0:512], sel_o[:, :],
                                 dninv2[:, 512:1024], start=False, stop=True)
                nc.vector.tensor_mul(att, stage2[:, :], rp[:, 0:512])
                pending = []

            # ---- remaining output projection (leftover + last block) ----
            for fn in filler:
                fn()
            filler = []
            for tt4 in range(4):
                for oc in range(2):
                    oproj_group(n_qb - 1, tt4, oc, in_loop=True)

    nc.finalize()
    return nc


_NC_CACHE = {}


def _get_nc(key=(N, C, NH)):
    if key not in _NC_CACHE:
        _NC_CACHE[key] = build_attention_kernel(*key)
    return _NC_CACHE[key]


def make_in_maps(x, Wq, bq, Wk, bk, Wv, bv, Wo):
    """Shard full inputs into 8 per-core input maps."""
    in_maps = []
    for c in range(NCORES):
        b, hg = divmod(c, 2)
        fs = slice(hg * F, (hg + 1) * F)
        in_maps.append({
            "xT": np.ascontiguousarray(x[b].T).astype(np.float16),
            "wq": np.ascontiguousarray(Wq[:, fs]).astype(np.float16),
            "wk": np.ascontiguousarray(Wk[:, fs]).astype(np.float16),
            "wv": np.ascontiguousarray(Wv[:, fs]).astype(np.float16),
            # prearranged so partition p holds [bias[a*128+p] for a in 4]
            # contiguously: one clean DMA descriptor per partition
            "bq": np.ascontiguousarray(
                bq[fs].reshape(F // 128, 128).T.reshape(F, 1)),
            "bk": np.ascontiguousarray(
                bk[fs].reshape(F // 128, 128).T.reshape(F, 1)),
            "bv": np.ascontiguousarray(bv[fs].reshape(1, F)).astype(np.float16),
            "wo": np.ascontiguousarray(Wo[fs, :]).astype(np.float16),
        })
    return in_maps


def kernel(x, Wq, bq, Wk, bk, Wv, bv, Wo, bo, **_unused):
    from concourse.bass_utils import run_bass_kernel_spmd

    arrs = [np.asarray(a, dtype=np.float32)
            for a in (x, Wq, bq, Wk, bk, Wv, bv, Wo, bo)]
    x, Wq, bq, Wk, bk, Wv, bv, Wo, bo = arrs

    nc = _get_nc()
    in_maps = make_in_maps(x, Wq, bq, Wk, bk, Wv, bv, Wo)
    res = run_bass_kernel_spmd(nc, in_maps, core_ids=list(range(NCORES)))

    out = np.empty((B, N, C), dtype=np.float32)
    for b in range(B):
        out[b] = (res.results[2 * b]["o_part"].astype(np.float32)
                  + res.results[2 * b + 1]["o_part"].astype(np.float32) + bo)
    return out


# revision 64
# speedup vs baseline: 1.1587x; 1.1587x over previous
"""Multi-head self-attention Trainium2 kernel (8 NeuronCores).

Problem: x[4, 2048, 1024], H=16 heads, D=64. Sharding: core c handles
batch b = c // 2 and head-group hg = c % 2 (8 heads = 512 features).

Per-core math (F = 512 core-local features, T = 2048 tokens, C = 1024),
all matmul operands fp16 (PE runs fp16 at 1 cycle/row like bf16, with
~8x the mantissa), fp32 PSUM accumulation:

  QT = (Wq_s.T @ x_b.T) + bq_s          [F, T]   (feature-major)
  KT = same with Wk_s                    [F, T]
  V65 = [x_b @ Wv_s + bv_s | ones]       [T, 8*(64+1)] interleaved per head
  per (head pair, 512-query block):
    for each key tile kt (128 keys):
      scT[key, q] = KT_h.T @ QT_h        (two heads packed into the PE
                                          array via partition offsets 0/64)
      ex = exp(scT / 8)                  fp16 (no max subtraction: |s|<~2)
      pv[65, q] += V65_h[kt].T @ ex      (row 64 = softmax denominator)
    attnT_h[:, q] = pv[0:64] * (1/pv[64])  (recip on DVE, replicated
                                          across partitions via PE matmul)
  o_part = attnT.T @ Wo_s                [T, C]
Host: out[b] = o_part[2b] + o_part[2b+1] + bo.

The attention loop is ACT(exp)-bound (~1.15us per 128x1024 tile), so
everything else is software-pipelined into its slack: the PE-side
normalize of block i is emitted mid-block i+1, and the projections for
later head pairs plus the output projection are fed from a filler
queue at two insertion points per block.
"""

import sys

import numpy as np

if "/opt/trn_rl_repo" not in sys.path:
    sys.path.insert(0, "/opt/trn_rl_repo")

import concourse.bass as bass
import concourse.mybir as mybir
import concourse.tile as tile
from concourse import bacc

F32 = mybir.dt.float32
F16 = mybir.dt.float16
AF = mybir.ActivationFunctionType

# Full-problem constants
B, N, C, H, D = 4, 2048, 1024, 16, 64
NCORES = 8
NH = 8          # heads per core
F = NH * D      # 512 core-local features
SCALE = 1.0 / 8.0  # 1/sqrt(D)


def build_attention_kernel(tok=N, cin=C, nh=NH):
    """Build the per-core Bass program. Returns the finalized Bass object."""
    f = nh * D
    assert tok % 512 == 0 and cin % 128 == 0 and f % 128 == 0
    c_t = cin // 128       # contraction tiles for projections (8)
    f_t = f // 128         # feature tiles = head pairs (4)
    t_t = tok // 128       # token tiles (16)
    n_qb = tok // 512      # query blocks (4)
    n_ct = cin // 128      # output-proj column tiles (8)

    nc = bacc.Bacc("TRN2", target_bir_lowering=False, debug=False,
                   num_devices=NCORES)

    xT = nc.dram_tensor("xT", [cin, tok], F16, kind="ExternalInput").ap()
    wq = nc.dram_tensor("wq", [cin, f], F16, kind="ExternalInput").ap()
    wk = nc.dram_tensor("wk", [cin, f], F16, kind="ExternalInput").ap()
    wv = nc.dram_tensor("wv", [cin, f], F16, kind="ExternalInput").ap()
    bq = nc.dram_tensor("bq", [f, 1], F32, kind="ExternalInput").ap()
    bk = nc.dram_tensor("bk", [f, 1], F32, kind="ExternalInput").ap()
    bv = nc.dram_tensor("bv", [1, f], F16, kind="ExternalInput").ap()
    wo = nc.dram_tensor("wo", [f, cin], F16, kind="ExternalInput").ap()
    o_part = nc.dram_tensor("o_part", [tok, cin], F16,
                            kind="ExternalOutput").ap()

    with tile.TileContext(nc) as tc:
        from contextlib import ExitStack
        with ExitStack() as ctx:
            # ---- pools ----
            p_sm = ctx.enter_context(tc.tile_pool(name="p_sm", bufs=1))
            p_x = ctx.enter_context(tc.tile_pool(name="p_x", bufs=1))
            p_w = ctx.enter_context(tc.tile_pool(name="p_w", bufs=1))
            p_qk = ctx.enter_context(tc.tile_pool(name="p_qk", bufs=1))
            p_v = ctx.enter_context(tc.tile_pool(name="p_v", bufs=1))
            p_at = ctx.enter_context(tc.tile_pool(name="p_at", bufs=1))
            p_ex = ctx.enter_context(tc.tile_pool(name="p_ex", bufs=3))
            p_dn = ctx.enter_context(tc.tile_pool(name="p_dn", bufs=4))
            p_st = ctx.enter_context(tc.tile_pool(name="p_st", bufs=4))
            p_os = ctx.enter_context(tc.tile_pool(name="p_os", bufs=3))
            # 6 banks of score ping-pong (3-deep: the ACT backlog absorbs
            # the inserted oproj/normalize PE work) + 2 banks of pv
            # accumulators (1-block-deep: the single staging copy frees
            # them fast enough)
            ps_sc = ctx.enter_context(
                tc.tile_pool(name="ps_sc", bufs=3, space="PSUM"))
            ps_pv = ctx.enter_context(
                tc.tile_pool(name="ps_pv", bufs=2, space="PSUM"))

            # ---- constants / biases ----
            ones64 = p_sm.tile([1, 64], F16, tag="ones64", name="ones64")
            nc.vector.memset(ones64[:, :], 1.0)
            # selectors for the one-shot denominator replicate: out rows
            # 0:64 get head-e's reciprocal, rows 64:128 get head-o's
            sel_e = p_sm.tile([1, 128], F16, tag="sele", name="sel_e")
            sel_o = p_sm.tile([1, 128], F16, tag="selo", name="sel_o")
            nc.vector.memset(sel_e[:, :], 0.0)
            nc.vector.memset(sel_e[:, 0:64], 1.0)
            nc.vector.memset(sel_o[:, :], 0.0)
            nc.vector.memset(sel_o[:, 64:128], 1.0)
            onestok = p_sm.tile([1, 128], F16, tag="onestok", name="onestok")
            nc.vector.memset(onestok[:, :], 1.0)
            # bq/bk arrive host-prearranged as [128, f_t] so the DMA is one
            # contiguous transfer (a strided rearrange here would emit
            # thousands of 4-byte packets and jam the queues at startup)
            bqs = p_sm.tile([128, f_t], F32, tag="bqs", name="bqs")
            bks = p_sm.tile([128, f_t], F32, tag="bks", name="bks")
            bvs = p_sm.tile([1, f], F16, tag="bvs", name="bvs")
            nc.sync.dma_start(bqs[:, :], bq.rearrange("(p a) o -> p (a o)", p=128))
            nc.sync.dma_start(bks[:, :], bk.rearrange("(p a) o -> p (a o)", p=128))
            nc.sync.dma_start(bvs[:, :], bv[:, :])

            # ---- weight + x loads (DMA, overlapped with compute) ----
            wk_s = [p_w.tile([128, f], F16, tag=f"wk{i}", name=f"wk_s{i}")
                    for i in range(c_t)]
            wv_s = [p_w.tile([128, f], F16, tag=f"wv{i}", name=f"wv_s{i}")
                    for i in range(c_t)]
            wq_s = [p_w.tile([128, f], F16, tag=f"wq{i}", name=f"wq_s{i}")
                    for i in range(c_t)]
            xs = [p_x.tile([128, tok], F16, tag=f"x{i}", name=f"xs{i}")
                  for i in range(c_t)]
            wo_s = [p_w.tile([128, cin], F16, tag=f"wo{i}", name=f"wo_s{i}")
                    for i in range(f_t)]
            # interleave the first K-projection group's inputs (wk + x
            # token-chunk 0) so it can start as early as possible; wv
            # before the x tail so V projection isn't the last to unblock
            for i in range(c_t):
                nc.sync.dma_start(wk_s[i][:, :], wk[i * 128:(i + 1) * 128, :])
                nc.sync.dma_start(xs[i][:, 0:512], xT[i * 128:(i + 1) * 128, 0:512])
            for tch in range(1, tok // 512):
                ts = slice(tch * 512, (tch + 1) * 512)
                for i in range(c_t):
                    nc.sync.dma_start(xs[i][:, ts], xT[i * 128:(i + 1) * 128, ts])
                    if tch == 1:
                        nc.sync.dma_start(wv_s[i][:, :],
                                          wv[i * 128:(i + 1) * 128, :])
            for i in range(c_t):
                nc.sync.dma_start(wq_s[i][:, :], wq[i * 128:(i + 1) * 128, :])
            for i in range(f_t):
                nc.sync.dma_start(wo_s[i][:, :], wo[i * 128:(i + 1) * 128, :])

            # ---- persistent activations ----
            KT = [p_qk.tile([128, tok], F16, tag=f"kt{i}", name=f"KT{i}")
                  for i in range(f_t)]
            QT = [p_qk.tile([128, tok], F16, tag=f"qt{i}", name=f"QT{i}")
                  for i in range(f_t)]
            # V tiles padded to 128 weight columns per head (64 values +
            # ones column at 64 + don't-care) so LDWEIGHTS gets FWL
            V65 = [p_v.tile([128, nh * 128], F16, tag=f"v{i}", name=f"V65_{i}")
                   for i in range(t_t)]
            attnT = [p_at.tile([128, tok], F16, tag=f"at{i}", name=f"attnT{i}")
                     for i in range(f_t)]

            # ---- emission helpers ----
            def qk_group(w_s, dst, bias, ft, tch):
                """One projection group: 512 tokens x 128 features of Q or K."""
                ts = slice(tch * 512, (tch + 1) * 512)
                ps = ps_sc.tile([128, 1024], F32, tag="sc",
                                name=f"psqk_{dst[ft].name}_{tch}")
                for i in range(c_t):
                    nc.tensor.matmul(
                        ps[:, 0:512],
                        w_s[i][:, ft * 128:(ft + 1) * 128],
                        xs[i][:, ts],
                        start=(i == 0), stop=(i == c_t - 1))
                nc.vector.tensor_scalar_add(
                    dst[ft][:, ts], ps[:, 0:512], bias[:, ft:ft + 1])

            def v_group(gt):
                """One V group: 128 tokens, all 512 features + ones col."""
                tsl = slice(gt * 128, (gt + 1) * 128)
                psv = ps_sc.tile([128, 1024], F32, tag="sc", name=f"psv{gt}")
                for i in range(c_t):
                    nc.tensor.matmul(
                        psv[:, 0:512], xs[i][:, tsl], wv_s[i][:, :],
                        start=(i == 0), stop=False)
                nc.tensor.matmul(psv[:, 0:512], onestok[:, :], bvs[:, :],
                                 start=False, stop=True)
                v_dst = V65[gt].rearrange("p (h e) -> p h e", e=128)
                nc.vector.tensor_copy(
                    v_dst[:, :, 0:64],
                    psv[:, 0:512].rearrange("p (h e) -> p h e", e=64)[:, :, :])

            def oproj_group(qb, tt4, oc, in_loop=False):
                """Output projection for 128 tokens x 512 out-channels.

                Allocates from the sc pool: mid-block the DVE is idle, so
                the drain copy frees the slot in ~0.7us. (The pv pool must
                NOT be used here — its 2-block rotation depth is what
                keeps the next block's pv accumulators from waiting on the
                previous block's DVE normalize chain.)
                """
                tt = qb * 4 + tt4
                tsl = slice(tt * 128, (tt + 1) * 128)
                osl = slice(oc * 512, (oc + 1) * 512)
                po = ps_sc.tile([128, 1024], F32, tag="sc",
                                name=f"po{tt}_{oc}")
                posl = po[:, 0:512]
                for i in range(f_t):
                    nc.tensor.matmul(posl, attnT[i][:, tsl],
                                     wo_s[i][:, osl],
                                     start=(i == 0), stop=(i == f_t - 1))
                ob = p_os.tile([128, 512], F16, tag="os", name=f"ob{tt}_{oc}")
                nc.vector.tensor_copy(ob[:, :], posl)
                nc.sync.dma_start(o_part[tsl, osl], ob[:, :])

            def oproj_group2(qb, u1, u2):
                """Two interleaved output-projection units (same trick as
                qk_group2: alternating banks pipeline the weight loads)."""
                slices = []
                pos = []
                for j, (tt4, oc) in enumerate((u1, u2)):
                    tt = qb * 4 + tt4
                    tsl = slice(tt * 128, (tt + 1) * 128)
                    osl = slice(oc * 512, (oc + 1) * 512)
                    po = ps_sc.tile([128, 1024], F32, tag="sc",
                                    name=f"po2_{tt}_{oc}")
                    slices.append((tsl, osl, tt, oc))
                    pos.append(po)
                for i in range(f_t):
                    for (tsl, osl, _, _), po in zip(slices, pos):
                        nc.tensor.matmul(po[:, 0:512], attnT[i][:, tsl],
                                         wo_s[i][:, osl],
                                         start=(i == 0), stop=(i == f_t - 1))
                for (tsl, osl, tt, oc), po in zip(slices, pos):
                    ob = p_os.tile([128, 512], F16, tag="os",
                                   name=f"ob2_{tt}_{oc}")
                    nc.vector.tensor_copy(ob[:, :], po[:, 0:512])
                    nc.sync.dma_start(o_part[tsl, osl], ob[:, :])

            # ---- PE warm-up: junk matmuls while input DMAs are in
            # flight, so the HAM clock gate reaches 2.4 GHz before the
            # real work starts ----
            junk = p_sm.tile([128, 512], F16, tag="junk", name="junk")
            nc.vector.memset(junk[:, :], 0.5)
            ps_junk = ps_sc.tile([128, 1024], F32, tag="sc", name="ps_junk")
            for _ in range(30):
                nc.tensor.matmul(ps_junk[:, 0:512], junk[:, 0:128],
                                 junk[:, :], start=True, stop=True)

            def qk_group2(w_s, dst, bias, fa, fb, tch):
                """Two interleaved projection groups: consecutive matmuls
                alternate PSUM banks and weight sets, so each weight load
                and pipe drain hides under the other group's stream."""
                ts = slice(tch * 512, (tch + 1) * 512)
                psA = ps_sc.tile([128, 1024], F32, tag="sc",
                                 name=f"psqkA_{dst[fa].name}_{tch}")
                psB = ps_sc.tile([128, 1024], F32, tag="sc",
                                 name=f"psqkB_{dst[fb].name}_{tch}")
                for i in range(c_t):
                    nc.tensor.matmul(
                        psA[:, 0:512], w_s[i][:, fa * 128:(fa + 1) * 128],
                        xs[i][:, ts], start=(i == 0), stop=(i == c_t - 1))
                    nc.tensor.matmul(
                        psB[:, 0:512], w_s[i][:, fb * 128:(fb + 1) * 128],
                        xs[i][:, ts], start=(i == 0), stop=(i == c_t - 1))
                nc.vector.tensor_scalar_add(
                    dst[fa][:, ts], psA[:, 0:512], bias[:, fa:fa + 1])
                nc.vector.tensor_scalar_add(
                    dst[fb][:, ts], psB[:, 0:512], bias[:, fb:fb + 1])

            def v_group2(ga, gb):
                psAB = []
                for g in (ga, gb):
                    ps = ps_sc.tile([128, 1024], F32, tag="sc",
                                    name=f"psv{g}")
                    psAB.append(ps)
                for i in range(c_t):
                    for g, ps in zip((ga, gb), psAB):
                        tsl = slice(g * 128, (g + 1) * 128)
                        nc.tensor.matmul(
                            ps[:, 0:512], xs[i][:, tsl], wv_s[i][:, :],
                            start=(i == 0), stop=False)
                for g, ps in zip((ga, gb), psAB):
                    nc.tensor.matmul(ps[:, 0:512], onestok[:, :], bvs[:, :],
                                     start=False, stop=True)
                for g, ps in zip((ga, gb), psAB):
                    v_dst = V65[g].rearrange("p (h e) -> p h e", e=128)
                    nc.vector.tensor_copy(
                        v_dst[:, :, 0:64],
                        ps[:, 0:512].rearrange("p (h e) -> p h e",
                                               e=64)[:, :, :])

            # ---- upfront phase 1: all projections, ordered to match DMA
            # arrival (K token-chunk-major first, then V, then Q) ----
            for gt in range(t_t):
                nc.vector.memset(V65[gt][:, :], 1.0)
            for tch in range(n_qb):
                qk_group2(wk_s, KT, bks, 0, 1, tch)
                qk_group2(wk_s, KT, bks, 2, 3, tch)
            for gt in range(0, t_t, 2):
                v_group2(gt, gt + 1)
            # only query-block 0's Q upfront; the rest streams into the
            # early attention blocks' slack (3-deep sc pool absorbs it)
            qk_group2(wq_s, QT, bqs, 0, 1, 0)
            qk_group2(wq_s, QT, bqs, 2, 3, 0)
            q_filler = [(p, tch) for tch in range(1, n_qb)
                        for p in range(f_t)]

            # filler queue: small output-projection units (~1.2us each)
            # that fit inside the attention loop's PE slack
            filler = []

            def emit_fill():
                if filler:
                    filler.pop(0)()

            # ================= Phase 2: attention =================
            pending = []  # deferred PE-side normalize of the previous block

            for qb in range(n_qb):
                qsl = slice(qb * 512, (qb + 1) * 512)
                for pair in range(f_t):
                    he, ho = 2 * pair, 2 * pair + 1
                    pv_e = ps_pv.tile([128, 512], F32, tag="pv",
                                      name=f"pv{pair}_{qb}e")
                    pv_o = ps_pv.tile([128, 512], F32, tag="pv",
                                      name=f"pv{pair}_{qb}o")
                    for kt in range(t_t):
                        if kt == 2 and q_filler:
                            p2, t2 = q_filler.pop(0)
                            qk_group(wq_s, QT, bqs, p2, t2)
                        if kt == 6:
                            # PE-side normalize of the previous block: one
                            # sc-pool allocation holds both heads' 1/den
                            # replicas, so only a single rotation slot is
                            # coupled to the DVE multiplies
                            if pending:
                                att, stage2, dninv2 = pending.pop()
                                rp = ps_sc.tile([128, 1024], F32, tag="sc",
                                                name=f"rp{pair}_{qb}")
                                nc.tensor.matmul(rp[:, 0:512], sel_e[:, :],
                                                 dninv2[:, 0:512],
                                                 start=True, stop=False)
                                nc.tensor.matmul(rp[:, 0:512], sel_o[:, :],
                                                 dninv2[:, 512:1024],
                                                 start=False, stop=True)
                                nc.vector.tensor_mul(att, stage2[:, :],
                                                     rp[:, 0:512])
                                pending = []
                            # attnT for qb-1 became final once pair3's
                            # part B ran: queue its output projection as
                            # paired units (one pop per block)
                            if pair == 0 and qb >= 1:
                                for tt4 in range(4):
                                    filler.append(
                                        lambda q=qb - 1, t=tt4:
                                        oproj_group2(q, (t, 0), (t, 1)))
                        if kt == 10:
                            emit_fill()
                        ksl = slice(kt * 128, (kt + 1) * 128)
                        sc = ps_sc.tile([128, 1024], F32, tag="sc",
                                        name=f"sc{pair}_{qb}_{kt}")
                        nc.tensor.matmul(sc[:, 0:512],
                                         KT[pair][0:64, ksl],
                                         QT[pair][0:64, qsl],
                                         start=True, stop=True)
                        nc.tensor.matmul(sc[:, 512:1024],
                                         KT[pair][64:128, ksl],
                                         QT[pair][64:128, qsl],
                                         start=True, stop=True)
                        ex = p_ex.tile([128, 1024], F16, tag="ex",
                                       name=f"ex{pair}_{qb}_{kt}")
                        nc.scalar.activation(ex[:, :], sc[:, :], AF.Exp,
                                             scale=SCALE)
                        nc.tensor.matmul(pv_e[:, :],
                                         V65[kt][:, he * 128:he * 128 + 128],
                                         ex[:, 0:512],
                                         start=(kt == 0), stop=(kt == t_t - 1))
                        nc.tensor.matmul(pv_o[:, :],
                                         V65[kt][:, ho * 128:ho * 128 + 128],
                                         ex[:, 512:1024],
                                         start=(kt == 0), stop=(kt == t_t - 1))

                    # part A (DVE only): stage both heads' unnormalized attn
                    # into one [128,512] tile (frees the pv banks fast) and
                    # compute both reciprocals in a single [2,512] pass
                    stage2 = p_st.tile([128, 512], F32, tag="st",
                                       name=f"st_{pair}_{qb}")
                    dnr2 = p_dn.tile([1, 1024], F32, tag="dnr",
                                     name=f"dnr_{pair}_{qb}")
                    nc.vector.tensor_copy(stage2[0:64, :], pv_e[0:64, :])
                    nc.vector.tensor_copy(dnr2[:, 0:512], pv_e[64:65, :])
                    nc.vector.tensor_copy(stage2[64:128, :], pv_o[0:64, :])
                    nc.vector.tensor_copy(dnr2[:, 512:1024], pv_o[64:65, :])
                    dninv32 = p_dn.tile([1, 1024], F32, tag="dn32",
                                        name=f"dn32_{pair}_{qb}")
                    nc.vector.reciprocal_approx_fast(
                        out=dninv32[:, :], in_=dnr2[:, :])
                    dninv2 = p_dn.tile([1, 1024], F16, tag="dn",
                                       name=f"dn{pair}_{qb}")
                    nc.vector.tensor_copy(dninv2[:, :], dninv32[:, :])
                    pending.append((attnT[pair][:, qsl], stage2, dninv2))

            if pending:
                att, stage2, dninv2 = pending.pop()
                rp = ps_sc.tile([128, 1024], F32, tag="sc", name="rp_last")
                nc.tensor.matmul(rp[:, 0:512], sel_e[:, :],
                                 dninv2[:, 0:512], start=True, stop=False)
                nc.tensor.matmul(rp[:, 0:512], sel_o[:, :],
                                 dninv2[:, 512:1024], start=False, stop=True)
                nc.vector.tensor_mul(att, stage2[:, :], rp[:, 0:512])
                pending = []

            # ---- remaining output projection (leftover + last block) ----
            for fn in filler:
                fn()
            filler = []
            for tt4 in range(4):
                oproj_group2(n_qb - 1, (tt4, 0), (tt4, 1))

    nc.finalize()
    return nc


_NC_CACHE = {}


def _get_nc(key=(N, C, NH)):
    if key not in _NC_CACHE:
        _NC_CACHE[key] = build_attention_kernel(*key)
    return _NC_CACHE[key]


def make_in_maps(x, Wq, bq, Wk, bk, Wv, bv, Wo):
    """Shard full inputs into 8 per-core input maps."""
    in_maps = []
    for c in range(NCORES):
        b, hg = divmod(c, 2)
        fs = slice(hg * F, (hg + 1) * F)
        in_maps.append({
            "xT": np.ascontiguousarray(x[b].T).astype(np.float16),
            "wq": np.ascontiguousarray(Wq[:, fs]).astype(np.float16),
            "wk": np.ascontiguousarray(Wk[:, fs]).astype(np.float16),
            "wv": np.ascontiguousarray(Wv[:, fs]).astype(np.float16),
            # prearranged so partition p holds [bias[a*128+p] for a in 4]
            # contiguously: one clean DMA descriptor per partition
            "bq": np.ascontiguousarray(
                bq[fs].reshape(F // 128, 128).T.reshape(F, 1)),
            "bk": np.ascontiguousarray(
                bk[fs].reshape(F // 128, 128).T.reshape(F, 1)),
            "bv": np.ascontiguousarray(bv[fs].reshape(1, F)).astype(np.float16),
            "wo": np.ascontiguousarray(Wo[fs, :]).astype(np.float16),
        })
    return in_maps


def kernel(x, Wq, bq, Wk, bk, Wv, bv, Wo, bo, **_unused):
    from concourse.bass_utils import run_bass_kernel_spmd

    arrs = [np.asarray(a, dtype=np.float32)
            for a in (x, Wq, bq, Wk, bk, Wv, bv, Wo, bo)]
    x, Wq, bq, Wk, bk, Wv, bv, Wo, bo = arrs

    nc = _get_nc()
    in_maps = make_in_maps(x, Wq, bq, Wk, bk, Wv, bv, Wo)
    res = run_bass_kernel_spmd(nc, in_maps, core_ids=list(range(NCORES)))

    out = np.empty((B, N, C), dtype=np.float32)
    for b in range(B):
        out[b] = (res.results[2 * b]["o_part"].astype(np.float32)
                  + res.results[2 * b + 1]["o_part"].astype(np.float32) + bo)
    return out


# revision 65
# speedup vs baseline: 1.1945x; 1.0309x over previous
"""Multi-head self-attention Trainium2 kernel (8 NeuronCores).

Problem: x[4, 2048, 1024], H=16 heads, D=64. Sharding: core c handles
batch b = c // 2 and head-group hg = c % 2 (8 heads = 512 features).

Per-core math (F = 512 core-local features, T = 2048 tokens, C = 1024),
all matmul operands fp16 (PE runs fp16 at 1 cycle/row like bf16, with
~8x the mantissa), fp32 PSUM accumulation:

  QT = (Wq_s.T @ x_b.T) + bq_s          [F, T]   (feature-major)
  KT = same with Wk_s                    [F, T]
  V65 = [x_b @ Wv_s + bv_s | ones]       [T, 8*(64+1)] interleaved per head
  per (head pair, 512-query block):
    for each key tile kt (128 keys):
      scT[key, q] = KT_h.T @ QT_h        (two heads packed into the PE
                                          array via partition offsets 0/64)
      ex = exp(scT / 8)                  fp16 (no max subtraction: |s|<~2)
      pv[65, q] += V65_h[kt].T @ ex      (row 64 = softmax denominator)
    attnT_h[:, q] = pv[0:64] * (1/pv[64])  (recip on DVE, replicated
                                          across partitions via PE matmul)
  o_part = attnT.T @ Wo_s                [T, C]
Host: out[b] = o_part[2b] + o_part[2b+1] + bo.

The attention loop is ACT(exp)-bound (~1.15us per 128x1024 tile), so
everything else is software-pipelined into its slack: the PE-side
normalize of block i is emitted mid-block i+1, and the projections for
later head pairs plus the output projection are fed from a filler
queue at two insertion points per block.
"""

import sys

import numpy as np

if "/opt/trn_rl_repo" not in sys.path:
    sys.path.insert(0, "/opt/trn_rl_repo")

import concourse.bass as bass
import concourse.mybir as mybir
import concourse.tile as tile
from concourse import bacc

F32 = mybir.dt.float32
F16 = mybir.dt.float16
AF = mybir.ActivationFunctionType

# Full-problem constants
B, N, C, H, D = 4, 2048, 1024, 16, 64
NCORES = 8
NH = 8          # heads per core
F = NH * D      # 512 core-local features
SCALE = 1.0 / 8.0  # 1/sqrt(D)


def build_attention_kernel(tok=N, cin=C, nh=NH):
    """Build the per-core Bass program. Returns the finalized Bass object."""
    f = nh * D
    assert tok % 512 == 0 and cin % 128 == 0 and f % 128 == 0
    c_t = cin // 128       # contraction tiles for projections (8)
    f_t = f // 128         # feature tiles = head pairs (4)
    t_t = tok // 128       # token tiles (16)
    n_qb = tok // 512      # query blocks (4)
    n_ct = cin // 128      # output-proj column tiles (8)

    nc = bacc.Bacc("TRN2", target_bir_lowering=False, debug=False,
                   num_devices=NCORES)

    xT = nc.dram_tensor("xT", [cin, tok], F16, kind="ExternalInput").ap()
    wq = nc.dram_tensor("wq", [cin, f], F16, kind="ExternalInput").ap()
    wk = nc.dram_tensor("wk", [cin, f], F16, kind="ExternalInput").ap()
    wv = nc.dram_tensor("wv", [cin, f], F16, kind="ExternalInput").ap()
    bq = nc.dram_tensor("bq", [f, 1], F32, kind="ExternalInput").ap()
    bk = nc.dram_tensor("bk", [f, 1], F32, kind="ExternalInput").ap()
    bv = nc.dram_tensor("bv", [1, f], F16, kind="ExternalInput").ap()
    wo = nc.dram_tensor("wo", [f, cin], F16, kind="ExternalInput").ap()
    o_part = nc.dram_tensor("o_part", [tok, cin], F16,
                            kind="ExternalOutput").ap()

    with tile.TileContext(nc) as tc:
        from contextlib import ExitStack
        with ExitStack() as ctx:
            # ---- pools ----
            p_sm = ctx.enter_context(tc.tile_pool(name="p_sm", bufs=1))
            p_x = ctx.enter_context(tc.tile_pool(name="p_x", bufs=1))
            p_w = ctx.enter_context(tc.tile_pool(name="p_w", bufs=1))
            p_qk = ctx.enter_context(tc.tile_pool(name="p_qk", bufs=1))
            p_v = ctx.enter_context(tc.tile_pool(name="p_v", bufs=1))
            p_at = ctx.enter_context(tc.tile_pool(name="p_at", bufs=1))
            p_ex = ctx.enter_context(tc.tile_pool(name="p_ex", bufs=3))
            p_dn = ctx.enter_context(tc.tile_pool(name="p_dn", bufs=4))
            p_st = ctx.enter_context(tc.tile_pool(name="p_st", bufs=4))
            p_os = ctx.enter_context(tc.tile_pool(name="p_os", bufs=3))
            # 6 banks of score ping-pong (3-deep: the ACT backlog absorbs
            # the inserted oproj/normalize PE work) + 2 banks of pv
            # accumulators (1-block-deep: the single staging copy frees
            # them fast enough)
            ps_sc = ctx.enter_context(
                tc.tile_pool(name="ps_sc", bufs=3, space="PSUM"))
            ps_pv = ctx.enter_context(
                tc.tile_pool(name="ps_pv", bufs=2, space="PSUM"))

            # ---- constants / biases ----
            ones64 = p_sm.tile([1, 64], F16, tag="ones64", name="ones64")
            nc.vector.memset(ones64[:, :], 1.0)
            # selectors for the one-shot denominator replicate: out rows
            # 0:64 get head-e's reciprocal, rows 64:128 get head-o's
            sel_e = p_sm.tile([1, 128], F16, tag="sele", name="sel_e")
            sel_o = p_sm.tile([1, 128], F16, tag="selo", name="sel_o")
            nc.vector.memset(sel_e[:, :], 0.0)
            nc.vector.memset(sel_e[:, 0:64], 1.0)
            nc.vector.memset(sel_o[:, :], 0.0)
            nc.vector.memset(sel_o[:, 64:128], 1.0)
            onestok = p_sm.tile([1, 128], F16, tag="onestok", name="onestok")
            nc.vector.memset(onestok[:, :], 1.0)
            # bq/bk arrive host-prearranged as [128, f_t] so the DMA is one
            # contiguous transfer (a strided rearrange here would emit
            # thousands of 4-byte packets and jam the queues at startup)
            bqs = p_sm.tile([128, f_t], F32, tag="bqs", name="bqs")
            bks = p_sm.tile([128, f_t], F32, tag="bks", name="bks")
            bvs = p_sm.tile([1, f], F16, tag="bvs", name="bvs")
            nc.sync.dma_start(bqs[:, :], bq.rearrange("(p a) o -> p (a o)", p=128))
            nc.sync.dma_start(bks[:, :], bk.rearrange("(p a) o -> p (a o)", p=128))
            nc.sync.dma_start(bvs[:, :], bv[:, :])

            # ---- weight + x loads (DMA, overlapped with compute) ----
            wk_s = [p_w.tile([128, f], F16, tag=f"wk{i}", name=f"wk_s{i}")
                    for i in range(c_t)]
            wv_s = [p_w.tile([128, f], F16, tag=f"wv{i}", name=f"wv_s{i}")
                    for i in range(c_t)]
            wq_s = [p_w.tile([128, f], F16, tag=f"wq{i}", name=f"wq_s{i}")
                    for i in range(c_t)]
            xs = [p_x.tile([128, tok], F16, tag=f"x{i}", name=f"xs{i}")
                  for i in range(c_t)]
            wo_s = [p_w.tile([128, cin], F16, tag=f"wo{i}", name=f"wo_s{i}")
                    for i in range(f_t)]
            # interleave the first K-projection group's inputs (wk + x
            # token-chunk 0) so it can start as early as possible; wv
            # before the x tail so V projection isn't the last to unblock
            for i in range(c_t):
                nc.sync.dma_start(wk_s[i][:, :], wk[i * 128:(i + 1) * 128, :])
                nc.sync.dma_start(xs[i][:, 0:512], xT[i * 128:(i + 1) * 128, 0:512])
            for tch in range(1, tok // 512):
                ts = slice(tch * 512, (tch + 1) * 512)
                for i in range(c_t):
                    nc.sync.dma_start(xs[i][:, ts], xT[i * 128:(i + 1) * 128, ts])
                    if tch == 1:
                        nc.sync.dma_start(wv_s[i][:, :],
                                          wv[i * 128:(i + 1) * 128, :])
            for i in range(c_t):
                nc.sync.dma_start(wq_s[i][:, :], wq[i * 128:(i + 1) * 128, :])
            for i in range(f_t):
                nc.sync.dma_start(wo_s[i][:, :], wo[i * 128:(i + 1) * 128, :])

            # ---- persistent activations ----
            KT = [p_qk.tile([128, tok], F16, tag=f"kt{i}", name=f"KT{i}")
                  for i in range(f_t)]
            QT = [p_qk.tile([128, tok], F16, tag=f"qt{i}", name=f"QT{i}")
                  for i in range(f_t)]
            # V tiles padded to 128 weight columns per head (64 values +
            # ones column at 64 + don't-care) so LDWEIGHTS gets FWL
            V65 = [p_v.tile([128, nh * 128], F16, tag=f"v{i}", name=f"V65_{i}")
                   for i in range(t_t)]
            attnT = [p_at.tile([128, tok], F16, tag=f"at{i}", name=f"attnT{i}")
                     for i in range(f_t)]

            # ---- emission helpers ----
            def qk_group(w_s, dst, bias, ft, tch):
                """One projection group: 512 tokens x 128 features of Q or K."""
                ts = slice(tch * 512, (tch + 1) * 512)
                ps = ps_sc.tile([128, 1024], F32, tag="sc",
                                name=f"psqk_{dst[ft].name}_{tch}")
                for i in range(c_t):
                    nc.tensor.matmul(
                        ps[:, 0:512],
                        w_s[i][:, ft * 128:(ft + 1) * 128],
                        xs[i][:, ts],
                        start=(i == 0), stop=(i == c_t - 1))
                nc.vector.tensor_scalar_add(
                    dst[ft][:, ts], ps[:, 0:512], bias[:, ft:ft + 1])

            def v_group(gt):
                """One V group: 128 tokens, all 512 features + ones col."""
                tsl = slice(gt * 128, (gt + 1) * 128)
                psv = ps_sc.tile([128, 1024], F32, tag="sc", name=f"psv{gt}")
                for i in range(c_t):
                    nc.tensor.matmul(
                        psv[:, 0:512], xs[i][:, tsl], wv_s[i][:, :],
                        start=(i == 0), stop=False)
                nc.tensor.matmul(psv[:, 0:512], onestok[:, :], bvs[:, :],
                                 start=False, stop=True)
                v_dst = V65[gt].rearrange("p (h e) -> p h e", e=128)
                nc.vector.tensor_copy(
                    v_dst[:, :, 0:64],
                    psv[:, 0:512].rearrange("p (h e) -> p h e", e=64)[:, :, :])

            def oproj_group(qb, tt4, oc, in_loop=False):
                """Output projection for 128 tokens x 512 out-channels.

                Allocates from the sc pool: mid-block the DVE is idle, so
                the drain copy frees the slot in ~0.7us. (The pv pool must
                NOT be used here — its 2-block rotation depth is what
                keeps the next block's pv accumulators from waiting on the
                previous block's DVE normalize chain.)
                """
                tt = qb * 4 + tt4
                tsl = slice(tt * 128, (tt + 1) * 128)
                osl = slice(oc * 512, (oc + 1) * 512)
                po = ps_sc.tile([128, 1024], F32, tag="sc",
                                name=f"po{tt}_{oc}")
                posl = po[:, 0:512]
                for i in range(f_t):
                    nc.tensor.matmul(posl, attnT[i][:, tsl],
                                     wo_s[i][:, osl],
                                     start=(i == 0), stop=(i == f_t - 1))
                ob = p_os.tile([128, 512], F16, tag="os", name=f"ob{tt}_{oc}")
                nc.vector.tensor_copy(ob[:, :], posl)
                nc.sync.dma_start(o_part[tsl, osl], ob[:, :])

            # ---- PE warm-up: junk matmuls while input DMAs are in
            # flight, so the HAM clock gate reaches 2.4 GHz before the
            # real work starts ----
            junk = p_sm.tile([128, 512], F16, tag="junk", name="junk")
            nc.vector.memset(junk[:, :], 0.5)
            ps_junk = ps_sc.tile([128, 1024], F32, tag="sc", name="ps_junk")
            for _ in range(30):
                nc.tensor.matmul(ps_junk[:, 0:512], junk[:, 0:128],
                                 junk[:, :], start=True, stop=True)

            def qk_group2(w_s, dst, bias, fa, fb, tch):
                """Two interleaved projection groups: consecutive matmuls
                alternate PSUM banks and weight sets, so each weight load
                and pipe drain hides under the other group's stream."""
                ts = slice(tch * 512, (tch + 1) * 512)
                psA = ps_sc.tile([128, 1024], F32, tag="sc",
                                 name=f"psqkA_{dst[fa].name}_{tch}")
                psB = ps_sc.tile([128, 1024], F32, tag="sc",
                                 name=f"psqkB_{dst[fb].name}_{tch}")
                for i in range(c_t):
                    nc.tensor.matmul(
                        psA[:, 0:512], w_s[i][:, fa * 128:(fa + 1) * 128],
                        xs[i][:, ts], start=(i == 0), stop=(i == c_t - 1))
                    nc.tensor.matmul(
                        psB[:, 0:512], w_s[i][:, fb * 128:(fb + 1) * 128],
                        xs[i][:, ts], start=(i == 0), stop=(i == c_t - 1))
                nc.vector.tensor_scalar_add(
                    dst[fa][:, ts], psA[:, 0:512], bias[:, fa:fa + 1])
                nc.vector.tensor_scalar_add(
                    dst[fb][:, ts], psB[:, 0:512], bias[:, fb:fb + 1])

            def v_group2(ga, gb):
                psAB = []
                for g in (ga, gb):
                    ps = ps_sc.tile([128, 1024], F32, tag="sc",
                                    name=f"psv{g}")
                    psAB.append(ps)
                for i in range(c_t):
                    for g, ps in zip((ga, gb), psAB):
                        tsl = slice(g * 128, (g + 1) * 128)
                        nc.tensor.matmul(
                            ps[:, 0:512], xs[i][:, tsl], wv_s[i][:, :],
                            start=(i == 0), stop=False)
                for g, ps in zip((ga, gb), psAB):
                    nc.tensor.matmul(ps[:, 0:512], onestok[:, :], bvs[:, :],
                                     start=False, stop=True)
                for g, ps in zip((ga, gb), psAB):
                    v_dst = V65[g].rearrange("p (h e) -> p h e", e=128)
                    nc.vector.tensor_copy(
                        v_dst[:, :, 0:64],
                        ps[:, 0:512].rearrange("p (h e) -> p h e",
                                               e=64)[:, :, :])

            # ---- upfront phase 1: all projections, ordered to match DMA
            # arrival (K token-chunk-major first, then V, then Q) ----
            for gt in range(t_t):
                nc.vector.memset(V65[gt][:, :], 1.0)
            for tch in range(n_qb):
                qk_group2(wk_s, KT, bks, 0, 1, tch)
                qk_group2(wk_s, KT, bks, 2, 3, tch)
            for gt in range(0, t_t, 2):
                v_group2(gt, gt + 1)
            # only query-block 0's Q upfront; the rest streams into the
            # early attention blocks' slack (3-deep sc pool absorbs it)
            qk_group2(wq_s, QT, bqs, 0, 1, 0)
            qk_group2(wq_s, QT, bqs, 2, 3, 0)
            q_filler = [(p, tch) for tch in range(1, n_qb)
                        for p in range(f_t)]

            # filler queue: small output-projection units (~1.2us each)
            # that fit inside the attention loop's PE slack
            filler = []

            def emit_fill():
                if filler:
                    filler.pop(0)()

            # ================= Phase 2: attention =================
            pending = []  # deferred PE-side normalize of the previous block

            for qb in range(n_qb):
                qsl = slice(qb * 512, (qb + 1) * 512)
                for pair in range(f_t):
                    he, ho = 2 * pair, 2 * pair + 1
                    pv_e = ps_pv.tile([128, 512], F32, tag="pv",
                                      name=f"pv{pair}_{qb}e")
                    pv_o = ps_pv.tile([128, 512], F32, tag="pv",
                                      name=f"pv{pair}_{qb}o")
                    for kt in range(t_t):
                        if kt == 2 and q_filler:
                            p2, t2 = q_filler.pop(0)
                            qk_group(wq_s, QT, bqs, p2, t2)
                        if kt == 6:
                            # PE-side normalize of the previous block: one
                            # sc-pool allocation holds both heads' 1/den
                            # replicas, so only a single rotation slot is
                            # coupled to the DVE multiplies
                            if pending:
                                att, stage2, dninv2 = pending.pop()
                                rp = ps_sc.tile([128, 1024], F32, tag="sc",
                                                name=f"rp{pair}_{qb}")
                                nc.tensor.matmul(rp[:, 0:512], sel_e[:, :],
                                                 dninv2[:, 0:512],
                                                 start=True, stop=False)
                                nc.tensor.matmul(rp[:, 0:512], sel_o[:, :],
                                                 dninv2[:, 512:1024],
                                                 start=False, stop=True)
                                nc.vector.tensor_mul(att, stage2[:, :],
                                                     rp[:, 0:512])
                                pending = []
                            # attnT for qb-1 became final once pair3's
                            # part B ran: queue its output projection
                            if pair == 0 and qb >= 1:
                                for tt4 in range(4):
                                    for oc in range(2):
                                        filler.append(
                                            lambda q=qb - 1, t=tt4, o=oc:
                                            oproj_group(q, t, o, in_loop=True))
                        if kt == 9:
                            emit_fill()
                        if kt == 13:
                            emit_fill()
                        ksl = slice(kt * 128, (kt + 1) * 128)
                        sc = ps_sc.tile([128, 1024], F32, tag="sc",
                                        name=f"sc{pair}_{qb}_{kt}")
                        nc.tensor.matmul(sc[:, 0:512],
                                         KT[pair][0:64, ksl],
                                         QT[pair][0:64, qsl],
                                         start=True, stop=True)
                        nc.tensor.matmul(sc[:, 512:1024],
                                         KT[pair][64:128, ksl],
                                         QT[pair][64:128, qsl],
                                         start=True, stop=True)
                        ex = p_ex.tile([128, 1024], F16, tag="ex",
                                       name=f"ex{pair}_{qb}_{kt}")
                        nc.scalar.activation(ex[:, :], sc[:, :], AF.Exp,
                                             scale=SCALE)
                        nc.tensor.matmul(pv_e[:, :],
                                         V65[kt][:, he * 128:he * 128 + 128],
                                         ex[:, 0:512],
                                         start=(kt == 0), stop=(kt == t_t - 1))
                        nc.tensor.matmul(pv_o[:, :],
                                         V65[kt][:, ho * 128:ho * 128 + 128],
                                         ex[:, 512:1024],
                                         start=(kt == 0), stop=(kt == t_t - 1))

                    # part A (DVE only): stage both heads' unnormalized attn
                    # into one [128,512] tile (frees the pv banks fast) and
                    # compute both reciprocals in a single [2,512] pass
                    stage2 = p_st.tile([128, 512], F32, tag="st",
                                       name=f"st_{pair}_{qb}")
                    dnr2 = p_dn.tile([1, 1024], F32, tag="dnr",
                                     name=f"dnr_{pair}_{qb}")
                    nc.vector.tensor_copy(stage2[0:64, :], pv_e[0:64, :])
                    nc.vector.tensor_copy(dnr2[:, 0:512], pv_e[64:65, :])
                    nc.vector.tensor_copy(stage2[64:128, :], pv_o[0:64, :])
                    nc.vector.tensor_copy(dnr2[:, 512:1024], pv_o[64:65, :])
                    dninv32 = p_dn.tile([1, 1024], F32, tag="dn32",
                                        name=f"dn32_{pair}_{qb}")
                    nc.vector.reciprocal_approx_fast(
                        out=dninv32[:, :], in_=dnr2[:, :])
                    dninv2 = p_dn.tile([1, 1024], F16, tag="dn",
                                       name=f"dn{pair}_{qb}")
                    nc.vector.tensor_copy(dninv2[:, :], dninv32[:, :])
                    pending.append((attnT[pair][:, qsl], stage2, dninv2))

            if pending:
                att, stage2, dninv2 = pending.pop()
                rp = ps_sc.tile([128, 1024], F32, tag="sc", name="rp_last")
                nc.tensor.matmul(rp[:, 0:512], sel_e[:, :],
                                 dninv2[:, 0:512], start=True, stop=False)
                nc.tensor.matmul(rp[:, 0:512], sel_o[:, :],
                                 dninv2[:, 512:1024], start=False, stop=True)
                nc.vector.tensor_mul(att, stage2[:, :], rp[:, 0:512])
                pending = []

            # ---- remaining output projection (leftover + last block) ----
            for fn in filler:
                fn()
            filler = []
            for tt4 in range(4):
                for oc in range(2):
                    oproj_group(n_qb - 1, tt4, oc, in_loop=True)

    nc.finalize()
    return nc


_NC_CACHE = {}


def _get_nc(key=(N, C, NH)):
    if key not in _NC_CACHE:
        _NC_CACHE[key] = build_attention_kernel(*key)
    return _NC_CACHE[key]


def make_in_maps(x, Wq, bq, Wk, bk, Wv, bv, Wo):
    """Shard full inputs into 8 per-core input maps."""
    in_maps = []
    for c in range(NCORES):
        b, hg = divmod(c, 2)
        fs = slice(hg * F, (hg + 1) * F)
        in_maps.append({
            "xT": np.ascontiguousarray(x[b].T).astype(np.float16),
            "wq": np.ascontiguousarray(Wq[:, fs]).astype(np.float16),
            "wk": np.ascontiguousarray(Wk[:, fs]).astype(np.float16),
            "wv": np.ascontiguousarray(Wv[:, fs]).astype(np.float16),
            # prearranged so partition p holds [bias[a*128+p] for a in 4]
            # contiguously: one clean DMA descriptor per partition
            "bq": np.ascontiguousarray(
                bq[fs].reshape(F // 128, 128).T.reshape(F, 1)),
            "bk": np.ascontiguousarray(
                bk[fs].reshape(F // 128, 128).T.reshape(F, 1)),
            "bv": np.ascontiguousarray(bv[fs].reshape(1, F)).astype(np.float16),
            "wo": np.ascontiguousarray(Wo[fs, :]).astype(np.float16),
        })
    return in_maps


def kernel(x, Wq, bq, Wk, bk, Wv, bv, Wo, bo, **_unused):
    from concourse.bass_utils import run_bass_kernel_spmd

    arrs = [np.asarray(a, dtype=np.float32)
            for a in (x, Wq, bq, Wk, bk, Wv, bv, Wo, bo)]
    x, Wq, bq, Wk, bk, Wv, bv, Wo, bo = arrs

    nc = _get_nc()
    in_maps = make_in_maps(x, Wq, bq, Wk, bk, Wv, bv, Wo)
    res = run_bass_kernel_spmd(nc, in_maps, core_ids=list(range(NCORES)))

    out = np.empty((B, N, C), dtype=np.float32)
    for b in range(B):
        out[b] = (res.results[2 * b]["o_part"].astype(np.float32)
                  + res.results[2 * b + 1]["o_part"].astype(np.float32) + bo)
    return out
